# revision 23
# baseline (speedup 1.0000x reference)
"""Trainium2 Bass kernel for CantorGlobalAttention (sparse attention).

Key algebraic fact: per (direction x, expert e, batch b) the scores are
rank-1 -- scores[p, k] = q[p] * kappa[k] -- so the attended output for
patch p is a smooth function of the single scalar q[p]:

    f(s) = sum_k e^{s*kappa_k} v_k / sum_k e^{s*kappa_k}

Instead of a 256 x 768 softmax per tuple, evaluate f EXACTLY at N=24
Chebyshev nodes s_i spanning [min q, max q], then polynomial-interpolate
to the 256 q_p values (host-computed cardinal weights L, one tiny
matmul). Validated: rel_norm ~4.4e-4 vs fp64 reference (budget 2e-2).

Per-core device pipeline (expert-pair sharding, pairs chosen so route
unions are minimal -- Cantor self-similarity gives e/e+5 identical
routes):
  - PE: node scores s_i*kappa_k as rank-1 matmuls ([5,128] lhsT of
    bf16 hi/lo kappa + ones rows, rhs carries node hi/lo + per-node
    max-shift rows), out PSUM [128k, 6*24].
  - ACT: one exp per tuple, [128,144] -> fp16 (8x less exp work than
    direct softmax).
  - PE: fnode = E^T @ v-slot per chunk -> PSUM [120(x,i), 129] per
    (e,b) group; 129th col (ones in v) = softmax denominator Z.
  - DVE: 1/Z, then f = F * (1/Z) -> fp16 [120, 128].
  - PE: one interp matmul per (e,b): out[d, p] = sum_{x,i} f[(x,i),d] *
    L[(x,i),p]; fusion over the 5 directions is folded into L (wts[x])
    and accumulates inside this single contraction.
  - V is deduplicated via an 8-slot-per-(x,b) layout (shared experts of
    the pair + per-side extras): 5.28 MB fp16 per core instead of
    7.9 MB duplicated.
"""

import os
import numpy as np
import ml_dtypes

import concourse.tile as tile
from concourse import bacc, mybir
from concourse.bass_utils import run_bass_kernel_spmd

F32 = mybir.dt.float32
BF16 = mybir.dt.bfloat16
FP16 = mybir.dt.float16
BF16_NP = ml_dtypes.bfloat16

NDIR = 5
E = 16
W = 3
D = 128
P = 256
B = 4
DEPTH = 8

N_CORES = 8
NN = 32                       # Chebyshev nodes per tuple
NG = 2 * B                    # (side, b) groups per core = 8
NT = NG * NDIR                # tuples per core = 40
NCH = 2 * W                   # key chunks per tuple = 6
NSLOT = 8                     # v slots per (x, b) block
VCOL = D + 1                  # slot cols (128 d + ones)

# expert pairs per core: chosen so |routes(eA) U routes(eB)| <= 4 and
# the pair shares >= 2 route experts (Cantor coords make e and e+5
# have identical routes).
PAIRS = [(0, 5), (1, 6), (2, 7), (3, 8), (4, 9), (10, 11), (12, 13), (14, 15)]
SLIST_A = [0, 1, 2, 3, 4, 5]  # chunk -> slot for side A
SLIST_B = [0, 1, 2, 3, 6, 7]  # chunk -> slot for side B


def _routes() -> np.ndarray:
    def cantor(pos: int) -> float:
        x = pos / max(1, E - 1)
        x = max(1e-06, min(x, 1.0 - 1e-06))
        val, factor = 0.0, 0.5
        for _ in range(DEPTH):
            x *= 3.0
            digit = int(x)
            x -= digit
            if digit == 2:
                val += factor
            factor *= 0.5
        return val

    coords = np.array([cantor(i) for i in range(E)], dtype=np.float32)
    routes = np.zeros((E, W), dtype=np.int32)
    for i in range(E):
        d = np.abs(coords - coords[i])
        routes[i] = np.sort(np.argsort(d, kind="stable")[:W])
    return routes


ROUTES = _routes()


def _build_program():
    nc = bacc.Bacc(None)

    vd = nc.dram_tensor("v", [NDIR * B, 128, NSLOT * VCOL], FP16,
                        kind="ExternalInput")
    kqld = nc.dram_tensor("kql", [5, NT * NCH * 128], BF16, kind="ExternalInput")
    kqrd = nc.dram_tensor("kqr", [5, NT * NN], BF16, kind="ExternalInput")
    # PE out base partitions must be in {0,32,64}: x=0,1,2 fill column
    # block 0 at partitions 0/32/64 (rows 96:128 zero), x=3,4 fill
    # column block 1 at partitions 0/32.
    lz1d = nc.dram_tensor("lz1", [128, NG * P], FP16, kind="ExternalInput")
    lz2d = nc.dram_tensor("lz2", [64, NG * P], FP16, kind="ExternalInput")
    od = nc.dram_tensor("o", [NG, 128, P], FP16, kind="ExternalOutput")

    with tile.TileContext(nc) as tc:
        with (
            tc.tile_pool(name="const", bufs=1) as const,
            tc.tile_pool(name="escore", bufs=4) as epool,
            tc.tile_pool(name="fsb", bufs=2) as fpool,
            tc.tile_pool(name="small", bufs=4) as spool,
            tc.tile_pool(name="psum_s", bufs=4, space="PSUM") as pscore,
            tc.tile_pool(name="psum_f", bufs=2, space="PSUM") as pfnode,
            tc.tile_pool(name="psum_o", bufs=2, space="PSUM") as pout,
        ):
            v_tile = const.tile([128, NDIR * B * NSLOT * VCOL], FP16)
            kql_tile = const.tile([5, NT * NCH * 128], BF16)
            kqr_tile = const.tile([5, NT * NN], BF16)
            lz1_tile = const.tile([128, NG * P], FP16)
            lz2_tile = const.tile([64, NG * P], FP16)

            # small operands first so compute can start early; kql in
            # chunks so the first tuples' scores can start ASAP
            nc.sync.dma_start(kqr_tile[:], kqrd[:])
            KQC = NT * NCH * 128 // 4
            for i in range(4):
                nc.sync.dma_start(
                    kql_tile[:, i * KQC : (i + 1) * KQC],
                    kqld[:, i * KQC : (i + 1) * KQC],
                )

            def dma_vblk(b):
                for x in range(NDIR):
                    blk = x * B + b
                    c0 = blk * NSLOT * VCOL
                    nc.sync.dma_start(
                        v_tile[:, c0 : c0 + NSLOT * VCOL], vd[blk]
                    )

            # v blocks in b-major order matching the group processing
            # order; lz is not needed until the first tail, so it rides
            # after batch 0's blocks
            def dma_lz(g):
                c0, c1 = g * P, (g + 1) * P
                nc.sync.dma_start(lz1_tile[:, c0:c1], lz1d[:, c0:c1])
                nc.sync.dma_start(lz2_tile[:, c0:c1], lz2d[:, c0:c1])

            dma_vblk(0)
            dma_lz(0)
            dma_lz(1)
            for b in range(1, B):
                dma_vblk(b)
                dma_lz(2 * b)
                dma_lz(2 * b + 1)

            # junk f_sb rows (96:128 of block 0) multiply zero lz rows;
            # memset the two rotating buffers once so they stay finite
            for _ in range(2):
                fs0 = fpool.tile([128, 2, D], FP16, tag="f_sb")
                nc.vector.memset(fs0[:], 0.0)

            # force the Exp ACT table load during startup
            scrap = const.tile([32, 8], F32)
            nc.vector.memset(scrap[:], 0.0)
            nc.scalar.activation(
                scrap[:], scrap[:], mybir.ActivationFunctionType.Exp
            )
            # PE p-state warmup (~3us of throwaway matmuls on zeros)
            warm = const.tile([32, 512], BF16)
            nc.gpsimd.memset(warm[:], 0.0)
            O = pout.tile([128, P], F32)
            for i in range(10):
                nc.tensor.matmul(
                    O[:], warm[0:32, 0:128], warm[0:32, 0:256],
                    start=True, stop=True,
                )

            def emit_scores_exp(g, x):
                b, side = g // 2, g % 2
                t = g * NDIR + x
                S = pscore.tile([128, NCH * NN], F32)
                for c in range(NCH):
                    k0 = (t * NCH + c) * 128
                    nc.tensor.matmul(
                        S[:, c * NN : (c + 1) * NN],
                        kql_tile[0:5, k0 : k0 + 128],
                        kqr_tile[0:5, t * NN : (t + 1) * NN],
                        start=True, stop=True,
                    )
                Ex = epool.tile([128, NCH * NN], FP16)
                nc.scalar.activation(
                    Ex[:], S[:], mybir.ActivationFunctionType.Exp
                )
                return Ex

            def emit_fnode(st):
                g, x, Ex, FT = st
                b, side = g // 2, g % 2
                slist = SLIST_A if side == 0 else SLIST_B
                blk = x * B + b
                if x < 3:
                    out_ap = FT[32 * x : 32 * x + NN, 0, :]
                else:
                    out_ap = FT[32 * (x - 3) : 32 * (x - 3) + NN, 1, :]
                for c in range(NCH):
                    sid = blk * NSLOT + slist[c]
                    nc.tensor.matmul(
                        out_ap,
                        Ex[:, c * NN : (c + 1) * NN],
                        v_tile[:, sid * VCOL : (sid + 1) * VCOL],
                        start=(c == 0), stop=(c == NCH - 1),
                    )

            def emit_tail_a(g, FT):
                rz = spool.tile([128, 2], F32)
                nc.vector.reciprocal(rz[:], FT[:, :, D])
                f_sb = fpool.tile([128, 2, D], FP16, tag="f_sb")
                nc.vector.tensor_scalar_mul(
                    f_sb[0:96, 0, :], FT[0:96, 0, 0:D], rz[0:96, 0:1]
                )
                nc.vector.tensor_scalar_mul(
                    f_sb[0:64, 1, :], FT[0:64, 1, 0:D], rz[0:64, 1:2]
                )
                return f_sb

            def emit_tail_b(g, f_sb):
                O = pout.tile([128, P], F32)
                nc.tensor.matmul(
                    O[:], f_sb[:, 0, :], lz1_tile[:, g * P : (g + 1) * P],
                    start=True, stop=False,
                )
                nc.tensor.matmul(
                    O[:], f_sb[0:64, 1, :], lz2_tile[:, g * P : (g + 1) * P],
                    start=False, stop=True,
                )
                Os = fpool.tile([128, P], FP16, tag=f"Os{g % 2}")
                nc.vector.tensor_scalar_add(Os[:], O[:], 0.0)
                nc.sync.dma_start(od[g], Os[:])

            # software pipeline: scores/exp run 2 tuples ahead of fnode so
            # ACT always has score input ready and PE interleaves cheaply
            variant = int(os.environ.get("KVARIANT", "2"))
            pending = []
            tails = []
            unit = 0

            def drain_one():
                nonlocal unit
                st = pending.pop(0)
                if variant >= 1:
                    emit_fnode(st)
                unit += 1
                if variant >= 2 and st[1] == NDIR - 1:
                    f_sb = emit_tail_a(st[0], st[3])
                    tails.append((st[0], f_sb, unit + 4))
                while tails and tails[0][2] <= unit:
                    gg, fsb, _ = tails.pop(0)
                    emit_tail_b(gg, fsb)

            for g in range(NG):
                FT = pfnode.tile([128, 2, VCOL], F32, tag=f"FT{g % 2}", bufs=1)
                for x in range(NDIR):
                    Ex = emit_scores_exp(g, x)
                    pending.append((g, x, Ex, FT))
                    while len(pending) > 3:
                        drain_one()
            while pending:
                drain_one()
            for gg, fsb, _ in tails:
                emit_tail_b(gg, fsb)

    nc.compile()
    return nc


_PROGRAM = None


def _program():
    global _PROGRAM
    if _PROGRAM is None:
        _PROGRAM = _build_program()
    return _PROGRAM


def _hi_lo(a):
    hi = a.astype(BF16_NP)
    lo = (a - hi.astype(np.float32)).astype(BF16_NP)
    return hi, lo


def _cheb_nodes(lo, hi, n):
    i = np.arange(n)
    t = np.cos((2 * i + 1) * np.pi / (2 * n))
    return (hi + lo) / 2 + (hi - lo) / 2 * t


def _cardinal_weights(nodes, xs):
    """L[p,i]: Lagrange cardinal functions at xs (barycentric, Cheb-1)."""
    n = len(nodes)
    w = np.array([(-1.0) ** i * np.sin((2 * i + 1) * np.pi / (2 * n))
                  for i in range(n)])
    diff = xs[:, None] - nodes[None, :]
    exact = np.isclose(diff, 0.0)
    diff_safe = np.where(exact, 1.0, diff)
    terms = w[None, :] / diff_safe
    L = terms / terms.sum(axis=1, keepdims=True)
    hit = exact.any(axis=1)
    L[hit] = np.where(exact[hit], 1.0, 0.0)
    return L


def _prep_core_inputs(core, Q_aff, K_aff, V, fac, wts):
    eA, eB = PAIRS[core]
    rA, rB = set(ROUTES[eA].tolist()), set(ROUTES[eB].tolist())
    shared = sorted(rA & rB)[:2]
    ownA = sorted(rA - set(shared))[0]
    ownB = sorted(rB - set(shared))[0]
    slot_expert = [shared[0], shared[0], shared[1], shared[1],
                   ownA, ownA, ownB, ownB]
    slot_half = [0, 1, 0, 1, 0, 1, 0, 1]

    v_host = np.empty((NDIR * B, 128, NSLOT * VCOL), dtype=np.float16)
    for x in range(NDIR):
        for b in range(B):
            blk = x * B + b
            for s in range(NSLOT):
                er, hf = slot_expert[s], slot_half[s]
                c0 = s * VCOL
                v_host[blk, :, c0 : c0 + D] = V[x, er, b,
                                                hf * 128 : (hf + 1) * 128, :]
                v_host[blk, :, c0 + D] = 1.0

    kql = np.zeros((5, NT * NCH * 128), dtype=BF16_NP)
    kqr = np.zeros((5, NT * NN), dtype=BF16_NP)
    lz1 = np.zeros((128, NG * P), dtype=np.float16)
    lz2 = np.zeros((64, NG * P), dtype=np.float16)

    for g in range(NG):
        b, side = g // 2, g % 2
        e = eA if side == 0 else eB
        slist = SLIST_A if side == 0 else SLIST_B
        for x in range(NDIR):
            t = g * NDIR + x
            q = Q_aff[x, e, b].astype(np.float64)
            kaps = []
            for c in range(NCH):
                er, hf = slot_expert[slist[c]], slot_half[slist[c]]
                kap = (K_aff[x, er, b, hf * 128 : (hf + 1) * 128]
                       .astype(np.float64) * fac[e, er]).astype(np.float32)
                kaps.append(kap)
                khi, klo = _hi_lo(kap)
                k0 = (t * NCH + c) * 128
                kql[0, k0 : k0 + 128] = khi
                kql[1, k0 : k0 + 128] = khi
                kql[2, k0 : k0 + 128] = klo
            kql[3, t * NCH * 128 : (t + 1) * NCH * 128] = 1.0
            kql[4, t * NCH * 128 : (t + 1) * NCH * 128] = 1.0

            s = _cheb_nodes(q.min(), q.max(), NN)
            kall = np.concatenate(kaps).astype(np.float64)
            m = np.maximum(s * kall.max(), s * kall.min())
            shi, slo = _hi_lo(s.astype(np.float32))
            mhi, mlo = _hi_lo(m.astype(np.float32))
            n0 = t * NN
            kqr[0, n0 : n0 + NN] = shi
            kqr[1, n0 : n0 + NN] = slo
            kqr[2, n0 : n0 + NN] = shi
            kqr[3, n0 : n0 + NN] = -mhi
            kqr[4, n0 : n0 + NN] = -mlo

            L = _cardinal_weights(s, q) * wts[x]
            LT = L.T.astype(np.float16)
            if x < 3:
                lz1[32 * x : 32 * (x + 1), g * P : (g + 1) * P] = LT
            else:
                x2 = x - 3
                lz2[32 * x2 : 32 * (x2 + 1), g * P : (g + 1) * P] = LT

    return {"v": v_host, "kql": kql, "kqr": kqr, "lz1": lz1, "lz2": lz2}


def kernel(Q_aff, K_aff, V, betas, temperature, fusion_weights):
    Q_aff = np.asarray(Q_aff, dtype=np.float32)
    K_aff = np.asarray(K_aff, dtype=np.float32)
    V = np.asarray(V, dtype=np.float32)
    betas = np.asarray(betas, dtype=np.float32)
    temperature = np.asarray(temperature, dtype=np.float32)
    fusion_weights = np.asarray(fusion_weights, dtype=np.float32)

    temp = abs(float(temperature[0])) + 1e-06
    sig = 1.0 / (1.0 + np.exp(-betas.astype(np.float64)))
    fac = np.empty((E, E), dtype=np.float64)
    for e in range(E):
        for er in range(E):
            fac[e, er] = (1.0 if er == e else sig[e, er]) / temp

    fw = fusion_weights.astype(np.float64)
    fw = np.exp(fw - fw.max())
    wts = (fw / fw.sum()).astype(np.float64)

    nc = _program()
    in_maps = [
        _prep_core_inputs(c, Q_aff, K_aff, V, fac, wts)
        for c in range(N_CORES)
    ]
    res = run_bass_kernel_spmd(nc, in_maps, list(range(N_CORES)))

    out = np.empty((B, E * P, D), dtype=np.float32)
    for c in range(N_CORES):
        o = res.results[c]["o"].astype(np.float32)  # [NG, 128 d, 256 p]
        for g in range(NG):
            b, side = g // 2, g % 2
            ge = PAIRS[c][side]
            out[b, ge * P : (ge + 1) * P, :] = o[g].T
    return out


# revision 24
# speedup vs baseline: 1.1098x; 1.1098x over previous
"""Trainium2 Bass kernel for CantorGlobalAttention (sparse attention).

Key algebraic fact: per (direction x, expert e, batch b) the scores are
rank-1 -- scores[p, k] = q[p] * kappa[k] -- so the attended output for
patch p is a smooth function of the single scalar q[p]:

    f(s) = sum_k e^{s*kappa_k} v_k / sum_k e^{s*kappa_k}

Instead of a 256 x 768 softmax per tuple, evaluate f EXACTLY at N=24
Chebyshev nodes s_i spanning [min q, max q], then polynomial-interpolate
to the 256 q_p values (host-computed cardinal weights L, one tiny
matmul). Validated: rel_norm ~4.4e-4 vs fp64 reference (budget 2e-2).

Per-core device pipeline (expert-pair sharding, pairs chosen so route
unions are minimal -- Cantor self-similarity gives e/e+5 identical
routes):
  - PE: node scores s_i*kappa_k as rank-1 matmuls ([5,128] lhsT of
    bf16 hi/lo kappa + ones rows, rhs carries node hi/lo + per-node
    max-shift rows), out PSUM [128k, 6*24].
  - ACT: one exp per tuple, [128,144] -> fp16 (8x less exp work than
    direct softmax).
  - PE: fnode = E^T @ v-slot per chunk -> PSUM [120(x,i), 129] per
    (e,b) group; 129th col (ones in v) = softmax denominator Z.
  - DVE: 1/Z, then f = F * (1/Z) -> fp16 [120, 128].
  - PE: one interp matmul per (e,b): out[d, p] = sum_{x,i} f[(x,i),d] *
    L[(x,i),p]; fusion over the 5 directions is folded into L (wts[x])
    and accumulates inside this single contraction.
  - V is deduplicated via an 8-slot-per-(x,b) layout (shared experts of
    the pair + per-side extras): 5.28 MB fp16 per core instead of
    7.9 MB duplicated.
"""

import os
import numpy as np
import ml_dtypes

import concourse.tile as tile
from concourse import bacc, mybir
from concourse.bass_utils import run_bass_kernel_spmd

F32 = mybir.dt.float32
BF16 = mybir.dt.bfloat16
FP16 = mybir.dt.float16
BF16_NP = ml_dtypes.bfloat16

NDIR = 5
E = 16
W = 3
D = 128
P = 256
B = 4
DEPTH = 8

N_CORES = 8
NN = 32                       # Chebyshev nodes per tuple
NG = 2 * B                    # (side, b) groups per core = 8
NT = NG * NDIR                # tuples per core = 40
NCH = 2 * W                   # key chunks per tuple = 6
NSLOT = 8                     # v slots per (x, b) block
VCOL = D + 1                  # slot cols (128 d + ones)

# expert pairs per core: chosen so |routes(eA) U routes(eB)| <= 4 and
# the pair shares >= 2 route experts (Cantor coords make e and e+5
# have identical routes).
PAIRS = [(0, 5), (1, 6), (2, 7), (3, 8), (4, 9), (10, 11), (12, 13), (14, 15)]
SLIST_A = [0, 1, 2, 3, 4, 5]  # chunk -> slot for side A
SLIST_B = [0, 1, 2, 3, 6, 7]  # chunk -> slot for side B


def _routes() -> np.ndarray:
    def cantor(pos: int) -> float:
        x = pos / max(1, E - 1)
        x = max(1e-06, min(x, 1.0 - 1e-06))
        val, factor = 0.0, 0.5
        for _ in range(DEPTH):
            x *= 3.0
            digit = int(x)
            x -= digit
            if digit == 2:
                val += factor
            factor *= 0.5
        return val

    coords = np.array([cantor(i) for i in range(E)], dtype=np.float32)
    routes = np.zeros((E, W), dtype=np.int32)
    for i in range(E):
        d = np.abs(coords - coords[i])
        routes[i] = np.sort(np.argsort(d, kind="stable")[:W])
    return routes


ROUTES = _routes()


def _build_program():
    nc = bacc.Bacc(None)

    vd = nc.dram_tensor("v", [NDIR * B, 128, NSLOT * VCOL], FP16,
                        kind="ExternalInput")
    kqld = nc.dram_tensor("kql", [5, NT * NCH * 128], BF16, kind="ExternalInput")
    kqrd = nc.dram_tensor("kqr", [5, NT * NN], BF16, kind="ExternalInput")
    # PE out base partitions must be in {0,32,64}: x=0,1,2 fill column
    # block 0 at partitions 0/32/64 (rows 96:128 zero), x=3,4 fill
    # column block 1 at partitions 0/32.
    lz1d = nc.dram_tensor("lz1", [128, NG * P], FP16, kind="ExternalInput")
    lz2d = nc.dram_tensor("lz2", [64, NG * P], FP16, kind="ExternalInput")
    od = nc.dram_tensor("o", [NG, 128, P], FP16, kind="ExternalOutput")

    with tile.TileContext(nc) as tc:
        with (
            tc.tile_pool(name="const", bufs=1) as const,
            tc.tile_pool(name="escore", bufs=4) as epool,
            tc.tile_pool(name="fsb", bufs=2) as fpool,
            tc.tile_pool(name="small", bufs=4) as spool,
            tc.tile_pool(name="psum_s", bufs=4, space="PSUM") as pscore,
            tc.tile_pool(name="psum_f", bufs=2, space="PSUM") as pfnode,
            tc.tile_pool(name="psum_o", bufs=2, space="PSUM") as pout,
        ):
            v_tile = const.tile([128, NDIR * B * NSLOT * VCOL], FP16)
            kql_tile = const.tile([5, NT * NCH * 128], BF16)
            kqr_tile = const.tile([5, NT * NN], BF16)
            lz1_tile = const.tile([128, NG * P], FP16)
            lz2_tile = const.tile([64, NG * P], FP16)

            # small operands first so compute can start early; kql in
            # chunks so the first tuples' scores can start ASAP
            nc.sync.dma_start(kqr_tile[:], kqrd[:])
            KQC = NT * NCH * 128 // 4
            for i in range(4):
                nc.sync.dma_start(
                    kql_tile[:, i * KQC : (i + 1) * KQC],
                    kqld[:, i * KQC : (i + 1) * KQC],
                )

            def dma_vblk(b):
                for x in range(NDIR):
                    blk = x * B + b
                    c0 = blk * NSLOT * VCOL
                    nc.sync.dma_start(
                        v_tile[:, c0 : c0 + NSLOT * VCOL], vd[blk]
                    )

            # v blocks in b-major order matching the group processing
            # order; lz is not needed until the first tail, so it rides
            # after batch 0's blocks
            dma_vblk(0)
            nc.sync.dma_start(lz1_tile[:], lz1d[:])
            nc.sync.dma_start(lz2_tile[:], lz2d[:])
            for b in range(1, B):
                dma_vblk(b)

            # junk f_sb rows (96:128 of block 0) multiply zero lz rows;
            # memset the two rotating buffers once so they stay finite
            for _ in range(2):
                fs0 = fpool.tile([128, 2, D], FP16, tag="f_sb")
                nc.vector.memset(fs0[:], 0.0)

            # force the Exp ACT table load during startup
            scrap = const.tile([32, 8], F32)
            nc.vector.memset(scrap[:], 0.0)
            nc.scalar.activation(
                scrap[:], scrap[:], mybir.ActivationFunctionType.Exp
            )
            # PE p-state warmup (~3us of throwaway matmuls on zeros)
            warm = const.tile([32, 512], BF16)
            nc.gpsimd.memset(warm[:], 0.0)
            O = pout.tile([128, P], F32)
            for i in range(10):
                nc.tensor.matmul(
                    O[:], warm[0:32, 0:128], warm[0:32, 0:256],
                    start=True, stop=True,
                )

            def emit_scores_exp(g, x):
                b, side = g // 2, g % 2
                t = g * NDIR + x
                S = pscore.tile([128, NCH * NN], F32)
                for c in range(NCH):
                    k0 = (t * NCH + c) * 128
                    nc.tensor.matmul(
                        S[:, c * NN : (c + 1) * NN],
                        kql_tile[0:5, k0 : k0 + 128],
                        kqr_tile[0:5, t * NN : (t + 1) * NN],
                        start=True, stop=True,
                    )
                Ex = epool.tile([128, NCH * NN], FP16)
                nc.scalar.activation(
                    Ex[:], S[:], mybir.ActivationFunctionType.Exp
                )
                return Ex

            def emit_fnode(st):
                g, x, Ex, FT = st
                b, side = g // 2, g % 2
                slist = SLIST_A if side == 0 else SLIST_B
                blk = x * B + b
                if x < 3:
                    out_ap = FT[32 * x : 32 * x + NN, 0, :]
                else:
                    out_ap = FT[32 * (x - 3) : 32 * (x - 3) + NN, 1, :]
                for c in range(NCH):
                    sid = blk * NSLOT + slist[c]
                    nc.tensor.matmul(
                        out_ap,
                        Ex[:, c * NN : (c + 1) * NN],
                        v_tile[:, sid * VCOL : (sid + 1) * VCOL],
                        start=(c == 0), stop=(c == NCH - 1),
                    )

            def emit_tail_a(g, FT):
                rz = spool.tile([128, 2], F32)
                nc.vector.reciprocal(rz[:], FT[:, :, D])
                f_sb = fpool.tile([128, 2, D], FP16, tag="f_sb")
                nc.vector.tensor_scalar_mul(
                    f_sb[0:96, 0, :], FT[0:96, 0, 0:D], rz[0:96, 0:1]
                )
                nc.vector.tensor_scalar_mul(
                    f_sb[0:64, 1, :], FT[0:64, 1, 0:D], rz[0:64, 1:2]
                )
                return f_sb

            def emit_tail_b(g, f_sb):
                O = pout.tile([128, P], F32)
                nc.tensor.matmul(
                    O[:], f_sb[:, 0, :], lz1_tile[:, g * P : (g + 1) * P],
                    start=True, stop=False,
                )
                nc.tensor.matmul(
                    O[:], f_sb[0:64, 1, :], lz2_tile[:, g * P : (g + 1) * P],
                    start=False, stop=True,
                )
                Os = fpool.tile([128, P], FP16, tag=f"Os{g % 2}")
                nc.vector.tensor_scalar_add(Os[:], O[:], 0.0)
                nc.sync.dma_start(od[g], Os[:])

            # software pipeline: scores/exp run 2 tuples ahead of fnode so
            # ACT always has score input ready and PE interleaves cheaply
            variant = int(os.environ.get("KVARIANT", "2"))
            pending = []
            tails = []
            unit = 0

            def drain_one():
                nonlocal unit
                st = pending.pop(0)
                if variant >= 1:
                    emit_fnode(st)
                unit += 1
                if variant >= 2 and st[1] == NDIR - 1:
                    f_sb = emit_tail_a(st[0], st[3])
                    tails.append((st[0], f_sb, unit + 4))
                while tails and tails[0][2] <= unit:
                    gg, fsb, _ = tails.pop(0)
                    emit_tail_b(gg, fsb)

            for g in range(NG):
                FT = pfnode.tile([128, 2, VCOL], F32, tag=f"FT{g % 2}", bufs=1)
                for x in range(NDIR):
                    Ex = emit_scores_exp(g, x)
                    pending.append((g, x, Ex, FT))
                    while len(pending) > 3:
                        drain_one()
            while pending:
                drain_one()
            for gg, fsb, _ in tails:
                emit_tail_b(gg, fsb)

    nc.compile()
    return nc


_PROGRAM = None


def _program():
    global _PROGRAM
    if _PROGRAM is None:
        _PROGRAM = _build_program()
    return _PROGRAM


def _hi_lo(a):
    hi = a.astype(BF16_NP)
    lo = (a - hi.astype(np.float32)).astype(BF16_NP)
    return hi, lo


def _cheb_nodes(lo, hi, n):
    i = np.arange(n)
    t = np.cos((2 * i + 1) * np.pi / (2 * n))
    return (hi + lo) / 2 + (hi - lo) / 2 * t


def _cardinal_weights(nodes, xs):
    """L[p,i]: Lagrange cardinal functions at xs (barycentric, Cheb-1)."""
    n = len(nodes)
    w = np.array([(-1.0) ** i * np.sin((2 * i + 1) * np.pi / (2 * n))
                  for i in range(n)])
    diff = xs[:, None] - nodes[None, :]
    exact = np.isclose(diff, 0.0)
    diff_safe = np.where(exact, 1.0, diff)
    terms = w[None, :] / diff_safe
    L = terms / terms.sum(axis=1, keepdims=True)
    hit = exact.any(axis=1)
    L[hit] = np.where(exact[hit], 1.0, 0.0)
    return L


def _prep_core_inputs(core, Q_aff, K_aff, V, fac, wts):
    eA, eB = PAIRS[core]
    rA, rB = set(ROUTES[eA].tolist()), set(ROUTES[eB].tolist())
    shared = sorted(rA & rB)[:2]
    ownA = sorted(rA - set(shared))[0]
    ownB = sorted(rB - set(shared))[0]
    slot_expert = [shared[0], shared[0], shared[1], shared[1],
                   ownA, ownA, ownB, ownB]
    slot_half = [0, 1, 0, 1, 0, 1, 0, 1]

    v_host = np.empty((NDIR * B, 128, NSLOT * VCOL), dtype=np.float16)
    for x in range(NDIR):
        for b in range(B):
            blk = x * B + b
            for s in range(NSLOT):
                er, hf = slot_expert[s], slot_half[s]
                c0 = s * VCOL
                v_host[blk, :, c0 : c0 + D] = V[x, er, b,
                                                hf * 128 : (hf + 1) * 128, :]
                v_host[blk, :, c0 + D] = 1.0

    kql = np.zeros((5, NT * NCH * 128), dtype=BF16_NP)
    kqr = np.zeros((5, NT * NN), dtype=BF16_NP)
    lz1 = np.zeros((128, NG * P), dtype=np.float16)
    lz2 = np.zeros((64, NG * P), dtype=np.float16)

    for g in range(NG):
        b, side = g // 2, g % 2
        e = eA if side == 0 else eB
        slist = SLIST_A if side == 0 else SLIST_B
        for x in range(NDIR):
            t = g * NDIR + x
            q = Q_aff[x, e, b].astype(np.float64)
            kaps = []
            for c in range(NCH):
                er, hf = slot_expert[slist[c]], slot_half[slist[c]]
                kap = (K_aff[x, er, b, hf * 128 : (hf + 1) * 128]
                       .astype(np.float64) * fac[e, er]).astype(np.float32)
                kaps.append(kap)
                khi, klo = _hi_lo(kap)
                k0 = (t * NCH + c) * 128
                kql[0, k0 : k0 + 128] = khi
                kql[1, k0 : k0 + 128] = khi
                kql[2, k0 : k0 + 128] = klo
            kql[3, t * NCH * 128 : (t + 1) * NCH * 128] = 1.0
            kql[4, t * NCH * 128 : (t + 1) * NCH * 128] = 1.0

            s = _cheb_nodes(q.min(), q.max(), NN)
            kall = np.concatenate(kaps).astype(np.float64)
            m = np.maximum(s * kall.max(), s * kall.min())
            shi, slo = _hi_lo(s.astype(np.float32))
            mhi, mlo = _hi_lo(m.astype(np.float32))
            n0 = t * NN
            kqr[0, n0 : n0 + NN] = shi
            kqr[1, n0 : n0 + NN] = slo
            kqr[2, n0 : n0 + NN] = shi
            kqr[3, n0 : n0 + NN] = -mhi
            kqr[4, n0 : n0 + NN] = -mlo

            L = _cardinal_weights(s, q) * wts[x]
            LT = L.T.astype(np.float16)
            if x < 3:
                lz1[32 * x : 32 * (x + 1), g * P : (g + 1) * P] = LT
            else:
                x2 = x - 3
                lz2[32 * x2 : 32 * (x2 + 1), g * P : (g + 1) * P] = LT

    return {"v": v_host, "kql": kql, "kqr": kqr, "lz1": lz1, "lz2": lz2}


def kernel(Q_aff, K_aff, V, betas, temperature, fusion_weights):
    Q_aff = np.asarray(Q_aff, dtype=np.float32)
    K_aff = np.asarray(K_aff, dtype=np.float32)
    V = np.asarray(V, dtype=np.float32)
    betas = np.asarray(betas, dtype=np.float32)
    temperature = np.asarray(temperature, dtype=np.float32)
    fusion_weights = np.asarray(fusion_weights, dtype=np.float32)

    temp = abs(float(temperature[0])) + 1e-06
    sig = 1.0 / (1.0 + np.exp(-betas.astype(np.float64)))
    fac = np.empty((E, E), dtype=np.float64)
    for e in range(E):
        for er in range(E):
            fac[e, er] = (1.0 if er == e else sig[e, er]) / temp

    fw = fusion_weights.astype(np.float64)
    fw = np.exp(fw - fw.max())
    wts = (fw / fw.sum()).astype(np.float64)

    nc = _program()
    in_maps = [
        _prep_core_inputs(c, Q_aff, K_aff, V, fac, wts)
        for c in range(N_CORES)
    ]
    res = run_bass_kernel_spmd(nc, in_maps, list(range(N_CORES)))

    out = np.empty((B, E * P, D), dtype=np.float32)
    for c in range(N_CORES):
        o = res.results[c]["o"].astype(np.float32)  # [NG, 128 d, 256 p]
        for g in range(NG):
            b, side = g // 2, g % 2
            ge = PAIRS[c][side]
            out[b, ge * P : (ge + 1) * P, :] = o[g].T
    return out


# revision 25
# speedup vs baseline: 1.1930x; 1.0750x over previous
"""Trainium2 Bass kernel for CantorGlobalAttention (sparse attention).

Key algebraic fact: per (direction x, expert e, batch b) the scores are
rank-1 -- scores[p, k] = q[p] * kappa[k] -- so the attended output for
patch p is a smooth function of the single scalar q[p]:

    f(s) = sum_k e^{s*kappa_k} v_k / sum_k e^{s*kappa_k}

Instead of a 256 x 768 softmax per tuple, evaluate f EXACTLY at N=24
Chebyshev nodes s_i spanning [min q, max q], then polynomial-interpolate
to the 256 q_p values (host-computed cardinal weights L, one tiny
matmul). Validated: rel_norm ~4.4e-4 vs fp64 reference (budget 2e-2).

Per-core device pipeline (expert-pair sharding, pairs chosen so route
unions are minimal -- Cantor self-similarity gives e/e+5 identical
routes):
  - PE: node scores s_i*kappa_k as rank-1 matmuls ([5,128] lhsT of
    bf16 hi/lo kappa + ones rows, rhs carries node hi/lo + per-node
    max-shift rows), out PSUM [128k, 6*24].
  - ACT: one exp per tuple, [128,144] -> fp16 (8x less exp work than
    direct softmax).
  - PE: fnode = E^T @ v-slot per chunk -> PSUM [120(x,i), 129] per
    (e,b) group; 129th col (ones in v) = softmax denominator Z.
  - DVE: 1/Z, then f = F * (1/Z) -> fp16 [120, 128].
  - PE: one interp matmul per (e,b): out[d, p] = sum_{x,i} f[(x,i),d] *
    L[(x,i),p]; fusion over the 5 directions is folded into L (wts[x])
    and accumulates inside this single contraction.
  - V is deduplicated via an 8-slot-per-(x,b) layout (shared experts of
    the pair + per-side extras): 5.28 MB fp16 per core instead of
    7.9 MB duplicated.
"""

import os
import numpy as np
import ml_dtypes

import concourse.tile as tile
from concourse import bacc, mybir
from concourse.bass_utils import run_bass_kernel_spmd

F32 = mybir.dt.float32
BF16 = mybir.dt.bfloat16
FP16 = mybir.dt.float16
BF16_NP = ml_dtypes.bfloat16

NDIR = 5
E = 16
W = 3
D = 128
P = 256
B = 4
DEPTH = 8

N_CORES = 8
NN = 32                       # Chebyshev nodes per tuple
NG = 2 * B                    # (side, b) groups per core = 8
NT = NG * NDIR                # tuples per core = 40
NCH = 2 * W                   # key chunks per tuple = 6
NSLOT = 8                     # v slots per (x, b) block
VCOL = D + 1                  # slot cols (128 d + ones)

# expert pairs per core: chosen so |routes(eA) U routes(eB)| <= 4 and
# the pair shares >= 2 route experts (Cantor coords make e and e+5
# have identical routes).
PAIRS = [(0, 5), (1, 6), (2, 7), (3, 8), (4, 9), (10, 11), (12, 13), (14, 15)]
SLIST_A = [0, 1, 2, 3, 4, 5]  # chunk -> slot for side A
SLIST_B = [0, 1, 2, 3, 6, 7]  # chunk -> slot for side B


def _routes() -> np.ndarray:
    def cantor(pos: int) -> float:
        x = pos / max(1, E - 1)
        x = max(1e-06, min(x, 1.0 - 1e-06))
        val, factor = 0.0, 0.5
        for _ in range(DEPTH):
            x *= 3.0
            digit = int(x)
            x -= digit
            if digit == 2:
                val += factor
            factor *= 0.5
        return val

    coords = np.array([cantor(i) for i in range(E)], dtype=np.float32)
    routes = np.zeros((E, W), dtype=np.int32)
    for i in range(E):
        d = np.abs(coords - coords[i])
        routes[i] = np.sort(np.argsort(d, kind="stable")[:W])
    return routes


ROUTES = _routes()


def _build_program():
    nc = bacc.Bacc(None)

    vd = nc.dram_tensor("v", [NDIR * B, 128, NSLOT * VCOL], FP16,
                        kind="ExternalInput")
    kqld = nc.dram_tensor("kql", [5, NT * NCH * 128], BF16, kind="ExternalInput")
    kqrd = nc.dram_tensor("kqr", [5, NT * NN], BF16, kind="ExternalInput")
    # PE out base partitions must be in {0,32,64}: x=0,1,2 fill column
    # block 0 at partitions 0/32/64 (rows 96:128 zero), x=3,4 fill
    # column block 1 at partitions 0/32.
    lz1d = nc.dram_tensor("lz1", [128, NG * P], FP16, kind="ExternalInput")
    lz2d = nc.dram_tensor("lz2", [64, NG * P], FP16, kind="ExternalInput")
    od = nc.dram_tensor("o", [NG, 128, P], FP16, kind="ExternalOutput")

    with tile.TileContext(nc) as tc:
        with (
            tc.tile_pool(name="const", bufs=1) as const,
            tc.tile_pool(name="escore", bufs=4) as epool,
            tc.tile_pool(name="fsb", bufs=2) as fpool,
            tc.tile_pool(name="small", bufs=4) as spool,
            tc.tile_pool(name="psum_s", bufs=4, space="PSUM") as pscore,
            tc.tile_pool(name="psum_f", bufs=2, space="PSUM") as pfnode,
            tc.tile_pool(name="psum_o", bufs=2, space="PSUM") as pout,
        ):
            v_tile = const.tile([128, NDIR * B * NSLOT * VCOL], FP16)
            kql_tile = const.tile([5, NT * NCH * 128], BF16)
            kqr_tile = const.tile([5, NT * NN], BF16)
            lz1_tile = const.tile([128, NG * P], FP16)
            lz2_tile = const.tile([64, NG * P], FP16)

            # small operands first so compute can start early
            nc.sync.dma_start(kql_tile[:], kqld[:])
            nc.sync.dma_start(kqr_tile[:], kqrd[:])

            def dma_vblk(b):
                for x in range(NDIR):
                    blk = x * B + b
                    c0 = blk * NSLOT * VCOL
                    nc.sync.dma_start(
                        v_tile[:, c0 : c0 + NSLOT * VCOL], vd[blk]
                    )

            # v blocks in b-major order matching the group processing
            # order; lz is not needed until the first tail, so it rides
            # after batch 0's blocks
            dma_vblk(0)
            nc.sync.dma_start(lz1_tile[:], lz1d[:])
            nc.sync.dma_start(lz2_tile[:], lz2d[:])
            for b in range(1, B):
                dma_vblk(b)

            # junk f_sb rows (96:128 of block 0) multiply zero lz rows;
            # memset the two rotating buffers once so they stay finite
            for _ in range(2):
                fs0 = fpool.tile([128, 2, D], FP16, tag="f_sb")
                nc.vector.memset(fs0[:], 0.0)

            # force the Exp ACT table load during startup
            scrap = const.tile([32, 8], F32)
            nc.vector.memset(scrap[:], 0.0)
            nc.scalar.activation(
                scrap[:], scrap[:], mybir.ActivationFunctionType.Exp
            )
            # PE p-state warmup (~3us of throwaway matmuls on zeros)
            warm = const.tile([32, 512], BF16)
            nc.gpsimd.memset(warm[:], 0.0)
            O = pout.tile([128, P], F32)
            for i in range(10):
                nc.tensor.matmul(
                    O[:], warm[0:32, 0:128], warm[0:32, 0:256],
                    start=True, stop=True,
                )

            def emit_scores_exp(g, x):
                b, side = g // 2, g % 2
                t = g * NDIR + x
                S = pscore.tile([128, NCH * NN], F32)
                for c in range(NCH):
                    k0 = (t * NCH + c) * 128
                    nc.tensor.matmul(
                        S[:, c * NN : (c + 1) * NN],
                        kql_tile[0:5, k0 : k0 + 128],
                        kqr_tile[0:5, t * NN : (t + 1) * NN],
                        start=True, stop=True,
                    )
                Ex = epool.tile([128, NCH * NN], FP16)
                nc.scalar.activation(
                    Ex[:], S[:], mybir.ActivationFunctionType.Exp
                )
                return Ex

            def emit_fnode(st):
                g, x, Ex, FT = st
                b, side = g // 2, g % 2
                slist = SLIST_A if side == 0 else SLIST_B
                blk = x * B + b
                if x < 3:
                    out_ap = FT[32 * x : 32 * x + NN, 0, :]
                else:
                    out_ap = FT[32 * (x - 3) : 32 * (x - 3) + NN, 1, :]
                for c in range(NCH):
                    sid = blk * NSLOT + slist[c]
                    nc.tensor.matmul(
                        out_ap,
                        Ex[:, c * NN : (c + 1) * NN],
                        v_tile[:, sid * VCOL : (sid + 1) * VCOL],
                        start=(c == 0), stop=(c == NCH - 1),
                    )

            def emit_tail_a(g, FT):
                rz = spool.tile([128, 2], F32)
                nc.vector.reciprocal(rz[:], FT[:, :, D])
                f_sb = fpool.tile([128, 2, D], FP16, tag="f_sb")
                nc.vector.tensor_scalar_mul(
                    f_sb[0:96, 0, :], FT[0:96, 0, 0:D], rz[0:96, 0:1]
                )
                nc.vector.tensor_scalar_mul(
                    f_sb[0:64, 1, :], FT[0:64, 1, 0:D], rz[0:64, 1:2]
                )
                return f_sb

            def emit_tail_b(g, f_sb):
                O = pout.tile([128, P], F32)
                nc.tensor.matmul(
                    O[:], f_sb[:, 0, :], lz1_tile[:, g * P : (g + 1) * P],
                    start=True, stop=False,
                )
                nc.tensor.matmul(
                    O[:], f_sb[0:64, 1, :], lz2_tile[:, g * P : (g + 1) * P],
                    start=False, stop=True,
                )
                Os = fpool.tile([128, P], FP16, tag=f"Os{g % 2}")
                nc.vector.tensor_scalar_add(Os[:], O[:], 0.0)
                nc.sync.dma_start(od[g], Os[:])

            # software pipeline: scores/exp run 2 tuples ahead of fnode so
            # ACT always has score input ready and PE interleaves cheaply
            variant = int(os.environ.get("KVARIANT", "2"))
            pending = []
            tails = []
            unit = 0

            def drain_one():
                nonlocal unit
                st = pending.pop(0)
                if variant >= 1:
                    emit_fnode(st)
                unit += 1
                if variant >= 2 and st[1] == NDIR - 1:
                    f_sb = emit_tail_a(st[0], st[3])
                    tails.append((st[0], f_sb, unit + 4))
                while tails and tails[0][2] <= unit:
                    gg, fsb, _ = tails.pop(0)
                    emit_tail_b(gg, fsb)

            for g in range(NG):
                FT = pfnode.tile([128, 2, VCOL], F32, tag=f"FT{g % 2}", bufs=1)
                for x in range(NDIR):
                    Ex = emit_scores_exp(g, x)
                    pending.append((g, x, Ex, FT))
                    while len(pending) > 3:
                        drain_one()
            while pending:
                drain_one()
            for gg, fsb, _ in tails:
                emit_tail_b(gg, fsb)

    nc.compile()
    return nc


_PROGRAM = None


def _program():
    global _PROGRAM
    if _PROGRAM is None:
        _PROGRAM = _build_program()
    return _PROGRAM


def _hi_lo(a):
    hi = a.astype(BF16_NP)
    lo = (a - hi.astype(np.float32)).astype(BF16_NP)
    return hi, lo


def _cheb_nodes(lo, hi, n):
    i = np.arange(n)
    t = np.cos((2 * i + 1) * np.pi / (2 * n))
    return (hi + lo) / 2 + (hi - lo) / 2 * t


def _cardinal_weights(nodes, xs):
    """L[p,i]: Lagrange cardinal functions at xs (barycentric, Cheb-1)."""
    n = len(nodes)
    w = np.array([(-1.0) ** i * np.sin((2 * i + 1) * np.pi / (2 * n))
                  for i in range(n)])
    diff = xs[:, None] - nodes[None, :]
    exact = np.isclose(diff, 0.0)
    diff_safe = np.where(exact, 1.0, diff)
    terms = w[None, :] / diff_safe
    L = terms / terms.sum(axis=1, keepdims=True)
    hit = exact.any(axis=1)
    L[hit] = np.where(exact[hit], 1.0, 0.0)
    return L


def _prep_core_inputs(core, Q_aff, K_aff, V, fac, wts):
    eA, eB = PAIRS[core]
    rA, rB = set(ROUTES[eA].tolist()), set(ROUTES[eB].tolist())
    shared = sorted(rA & rB)[:2]
    ownA = sorted(rA - set(shared))[0]
    ownB = sorted(rB - set(shared))[0]
    slot_expert = [shared[0], shared[0], shared[1], shared[1],
                   ownA, ownA, ownB, ownB]
    slot_half = [0, 1, 0, 1, 0, 1, 0, 1]

    v_host = np.empty((NDIR * B, 128, NSLOT * VCOL), dtype=np.float16)
    for x in range(NDIR):
        for b in range(B):
            blk = x * B + b
            for s in range(NSLOT):
                er, hf = slot_expert[s], slot_half[s]
                c0 = s * VCOL
                v_host[blk, :, c0 : c0 + D] = V[x, er, b,
                                                hf * 128 : (hf + 1) * 128, :]
                v_host[blk, :, c0 + D] = 1.0

    kql = np.zeros((5, NT * NCH * 128), dtype=BF16_NP)
    kqr = np.zeros((5, NT * NN), dtype=BF16_NP)
    lz1 = np.zeros((128, NG * P), dtype=np.float16)
    lz2 = np.zeros((64, NG * P), dtype=np.float16)

    for g in range(NG):
        b, side = g // 2, g % 2
        e = eA if side == 0 else eB
        slist = SLIST_A if side == 0 else SLIST_B
        for x in range(NDIR):
            t = g * NDIR + x
            q = Q_aff[x, e, b].astype(np.float64)
            kaps = []
            for c in range(NCH):
                er, hf = slot_expert[slist[c]], slot_half[slist[c]]
                kap = (K_aff[x, er, b, hf * 128 : (hf + 1) * 128]
                       .astype(np.float64) * fac[e, er]).astype(np.float32)
                kaps.append(kap)
                khi, klo = _hi_lo(kap)
                k0 = (t * NCH + c) * 128
                kql[0, k0 : k0 + 128] = khi
                kql[1, k0 : k0 + 128] = khi
                kql[2, k0 : k0 + 128] = klo
            kql[3, t * NCH * 128 : (t + 1) * NCH * 128] = 1.0
            kql[4, t * NCH * 128 : (t + 1) * NCH * 128] = 1.0

            s = _cheb_nodes(q.min(), q.max(), NN)
            kall = np.concatenate(kaps).astype(np.float64)
            m = np.maximum(s * kall.max(), s * kall.min())
            shi, slo = _hi_lo(s.astype(np.float32))
            mhi, mlo = _hi_lo(m.astype(np.float32))
            n0 = t * NN
            kqr[0, n0 : n0 + NN] = shi
            kqr[1, n0 : n0 + NN] = slo
            kqr[2, n0 : n0 + NN] = shi
            kqr[3, n0 : n0 + NN] = -mhi
            kqr[4, n0 : n0 + NN] = -mlo

            L = _cardinal_weights(s, q) * wts[x]
            LT = L.T.astype(np.float16)
            if x < 3:
                lz1[32 * x : 32 * (x + 1), g * P : (g + 1) * P] = LT
            else:
                x2 = x - 3
                lz2[32 * x2 : 32 * (x2 + 1), g * P : (g + 1) * P] = LT

    return {"v": v_host, "kql": kql, "kqr": kqr, "lz1": lz1, "lz2": lz2}


def kernel(Q_aff, K_aff, V, betas, temperature, fusion_weights):
    Q_aff = np.asarray(Q_aff, dtype=np.float32)
    K_aff = np.asarray(K_aff, dtype=np.float32)
    V = np.asarray(V, dtype=np.float32)
    betas = np.asarray(betas, dtype=np.float32)
    temperature = np.asarray(temperature, dtype=np.float32)
    fusion_weights = np.asarray(fusion_weights, dtype=np.float32)

    temp = abs(float(temperature[0])) + 1e-06
    sig = 1.0 / (1.0 + np.exp(-betas.astype(np.float64)))
    fac = np.empty((E, E), dtype=np.float64)
    for e in range(E):
        for er in range(E):
            fac[e, er] = (1.0 if er == e else sig[e, er]) / temp

    fw = fusion_weights.astype(np.float64)
    fw = np.exp(fw - fw.max())
    wts = (fw / fw.sum()).astype(np.float64)

    nc = _program()
    in_maps = [
        _prep_core_inputs(c, Q_aff, K_aff, V, fac, wts)
        for c in range(N_CORES)
    ]
    res = run_bass_kernel_spmd(nc, in_maps, list(range(N_CORES)))

    out = np.empty((B, E * P, D), dtype=np.float32)
    for c in range(N_CORES):
        o = res.results[c]["o"].astype(np.float32)  # [NG, 128 d, 256 p]
        for g in range(NG):
            b, side = g // 2, g % 2
            ge = PAIRS[c][side]
            out[b, ge * P : (ge + 1) * P, :] = o[g].T
    return out
